# revision 65
# baseline (speedup 1.0000x reference)
"""Masked 5x5 conv (PixelCNN 'A' mask) on 8 Trainium2 NeuronCores.

Problem (hardcoded): x[4,192,128,128] f32, weight[384,192,5,5] f32,
bias[384] f32, mask[4,1,128,128] i32.
out = where(window_any(mask), conv(x, weight*maskA) + bias, 0).

The 'A' causal mask keeps 12 of 25 taps: rows kh=0,1 fully, row kh=2 only
kw=0,1 -- i.e. every tap reads the current output row or rows above it.

Sharding: core c = (batch b = c//2, row-half = c%2). Each core computes one
batch's 64 output rows for all 384 out channels (3 M=128 chunks).

Per output tile [128 cout, 4 rows x 128 cols = 512] we accumulate 18 K=128
bf16 matmuls into one PSUM bank:
  - 12 taps x channel-chunk ci[0:128]  (from tile xa)
  - 5 tap-PAIRS x ci[128:192]          (from tile xb: lower 64 partitions =
    ci[128:192] data, upper 64 = same data shifted 1 col, so one K=128
    matmul covers two taps that differ by (0,+1))
  - 1 tap-pair (0,4)+(1,4) x ci[128:192] (tile xc: upper shifted one row)
Epilogue: one DVE scalar_tensor_tensor: out = (psum + bias) * valid.
"""

import numpy as np
import ml_dtypes

import concourse.bass as bass
import concourse.tile as tile
from concourse import mybir
from concourse.bass_utils import run_bass_kernel_spmd

B, CIN, COUT, H, W = 4, 192, 384, 128, 128
KH = KW = 5
PAD = 2
NCORES = 8
HHALF = 64          # output rows per core
NROWS = HHALF + 2   # input rows staged per core (2 above)
WP = W + 4          # padded width
FLAT = NROWS * WP   # 66*132 = 8712
RB = 4              # output rows per block
NBLK = HHALF // RB  # 16 blocks
NFREE = RB * W      # 512 = one PSUM bank of fp32

# Active taps of the 'A' mask, (kh, kw)
TAPS = [(0, 0), (0, 1), (0, 2), (0, 3), (0, 4),
        (1, 0), (1, 1), (1, 2), (1, 3), (1, 4),
        (2, 0), (2, 1)]
# ci[128:192] handled as pairs packed into K=128 matmuls.
# slab xb (upper shifted +1 element = +1 col): pairs differing by (0,1)
PAIRS_XB = [((0, 0), (0, 1)), ((0, 2), (0, 3)),
            ((1, 0), (1, 1)), ((1, 2), (1, 3)), ((2, 0), (2, 1))]
# slab xc (upper shifted +132 elements = +1 row): the leftover pair
PAIR_XC = ((0, 4), (1, 4))

BF16 = ml_dtypes.bfloat16


def _build_program():
    """Raw Bass (no Tile): this walrus build rejects instructions carrying
    more than ~1 embedded sync wait, so all synchronization is standalone
    wait_ge instructions with manually-managed semaphores.

    Schedule (per core, ~210us):
      - PE pre-warm: 14 dummy matmuls during the initial DMA wait flip the
        HAM clock gate to 2.4 GHz before the real stream begins.
      - Input DMAs stream in prioritized serialized waves (queues are
        ~45-90 GB/s each, ~358 GB/s aggregate).
      - Phase A runs the 12 xa-slots of tiles 0..7 as soon as the first
        weight/xa chunks land; phase B completes those tiles with the
        xb/xc pair slots once those tensors arrive; then steady state:
        18 K=128 matmuls per [128 cout x 512 spatial] PSUM tile.
      - DVE fuses (psum + bias) * valid into one scalar_tensor_tensor per
        tile, writing a bf16 staging buffer; outputs stream out in 2-tile
        chunks with a tapered, 2-way-split final chunk."""
    nc = bass.Bass()
    bf = mybir.dt.bfloat16
    f32 = mybir.dt.float32

    xa_d = nc.dram_tensor("xa", [128, FLAT], bf, kind="ExternalInput")
    xb_d = nc.dram_tensor("xb", [128, FLAT], bf, kind="ExternalInput")
    xc_d = nc.dram_tensor("xc", [128, FLAT], bf, kind="ExternalInput")
    wt_d = nc.dram_tensor("wt", [128, 18 * COUT], bf, kind="ExternalInput")
    bt_d = nc.dram_tensor("bt", [128, 3], f32, kind="ExternalInput")
    vt_d = nc.dram_tensor("vt", [128, HHALF * W], bf, kind="ExternalInput")
    out_d = nc.dram_tensor("out", [128, 3 * HHALF * W], bf, kind="ExternalOutput")

    NPS = 8           # psum banks in rotation
    PHA = 8           # tiles 0..PHA-1 run split-phase (xa first, xb/xc later)
    XA1 = 38 * WP     # xa chunk 1 covers input rows 0..37 (output blocks 0..7)
    OCH = 2           # out-DMA granularity: blocks per chunk
    NT = 3 * NBLK     # 48 tiles

    from contextlib import ExitStack
    with ExitStack() as ctx:
        xa_t = ctx.enter_context(nc.sbuf_tensor([128, FLAT], bf))
        xb_t = ctx.enter_context(nc.sbuf_tensor([128, FLAT], bf))
        xc_t = ctx.enter_context(nc.sbuf_tensor([128, FLAT], bf))
        wt_t = ctx.enter_context(nc.sbuf_tensor([128, 18 * COUT], bf))
        bt_t = ctx.enter_context(nc.sbuf_tensor([128, 3], f32))
        vt_t = ctx.enter_context(nc.sbuf_tensor([128, HHALF * W], bf))
        st_t = ctx.enter_context(nc.sbuf_tensor([128, 3 * HHALF * W], bf))
        ps_t = ctx.enter_context(nc.psum_tensor([128, NPS * NFREE], f32))
        da0 = ctx.enter_context(nc.semaphore("da0"))
        da1 = ctx.enter_context(nc.semaphore("da1"))
        da2 = ctx.enter_context(nc.semaphore("da2"))
        db1 = ctx.enter_context(nc.semaphore("db1"))
        db2 = ctx.enter_context(nc.semaphore("db2"))
        dc1 = ctx.enter_context(nc.semaphore("dc1"))
        dc2 = ctx.enter_context(nc.semaphore("dc2"))
        dwt1 = ctx.enter_context(nc.semaphore("dwt1"))
        dwt2 = ctx.enter_context(nc.semaphore("dwt2"))
        drest = ctx.enter_context(nc.semaphore("drest"))
        pes = ctx.enter_context(nc.semaphore("pes"))
        dve = ctx.enter_context(nc.semaphore("dve"))
        dout = ctx.enter_context(nc.semaphore("dout"))
        warm = ctx.enter_context(nc.semaphore("warm"))
        block = ctx.enter_context(nc.Block())
        xa_v = xa_t[:].rearrange("p (r c) -> p r c", c=WP)
        xb_v = xb_t[:].rearrange("p (r c) -> p r c", c=WP)
        xc_v = xc_t[:].rearrange("p (r c) -> p r c", c=WP)

        # (global weight-slot index, view, kh, kw)
        slots_a = [(s, xa_v, kh, kw) for s, (kh, kw) in enumerate(TAPS)]
        slots_bc = [(12 + i, xb_v, ta[0], ta[1])
                    for i, (ta, _tb) in enumerate(PAIRS_XB)]
        slots_bc += [(17, xc_v, PAIR_XC[0][0], PAIR_XC[0][1])]

        def emit_mms(tensor, k, sl, start, stop):
            m, blk = divmod(k, NBLK)
            j0 = blk * RB
            ps = ps_t[:, (k % NPS) * NFREE:(k % NPS + 1) * NFREE]
            n = len(sl)
            for i, (s, view, kh, kw) in enumerate(sl):
                mm = nc.tensor.matmul(
                    ps,
                    wt_t[:, s * COUT + m * 128: s * COUT + (m + 1) * 128],
                    view[:, j0 + kh: j0 + kh + RB, kw: kw + W],
                    start=(start and i == 0),
                    stop=(stop and i == n - 1),
                )
                if stop and i == n - 1:
                    mm.then_inc(pes, 1)

        @block.sync
        def _(sync):
            # DMA queues give ~45-90 GB/s per stream and ~358 GB/s aggregate,
            # so stream in prioritized serialized waves, each wave split
            # across a few queues. Wave 1a covers the very first matmuls.
            WT1 = 12 * COUT   # wt cols for the 12 xa slots
            XA0 = 14 * WP     # xa rows 0..13: blocks 0..2
            def split2(dst, src, lo, hi, sem):
                mid = ((lo + hi) // 2 // 4) * 4
                sync.dma_start(dst[:, lo:mid], src[:, lo:mid]).then_inc(sem, 16)
                sync.dma_start(dst[:, mid:hi], src[:, mid:hi]).then_inc(sem, 16)

            # wave 1a: wt slots 0..11 + xa rows 0..13, two chunks each
            # (3-way wt split measured slower -- stream contention)
            split2(wt_t, wt_d, 0, WT1, dwt1)
            split2(xa_t, xa_d, 0, XA0, da0)
            sync.wait_ge(dwt1, 32)
            sync.wait_ge(da0, 32)
            split2(xa_t, xa_d, XA0, XA1, da1)
            sync.wait_ge(da1, 32)
            # wave 2: phase-B inputs + DVE epilogue inputs; xb first
            split2(xb_t, xb_d, 0, XA1, db1)
            split2(xc_t, xc_d, 0, XA1, dc1)
            sync.dma_start(wt_t[:, WT1:], wt_d[:, WT1:]).then_inc(dwt2, 16)
            sync.dma_start(bt_t[:], bt_d[:]).then_inc(drest, 16)
            split2(vt_t, vt_d, 0, HHALF * W, drest)
            sync.wait_ge(db1, 32)
            sync.wait_ge(dc1, 32)
            # wave 3: steady-state remainders
            split2(xa_t, xa_d, XA1, FLAT, da2)
            split2(xb_t, xb_d, XA1, FLAT, db2)
            split2(xc_t, xc_d, XA1, FLAT, dc2)
            # output chunks of OCH tiles; the last two tiles go out singly
            # (the final DMA is the only one on the critical path, so the
            # smaller and more parallel it is, the shorter the tail)
            nch = NT // OCH
            ninc = 0
            for c in range(nch):
                lo, hi = c * OCH * NFREE, (c + 1) * OCH * NFREE
                if c == nch - 1:
                    # tile 46, then the two halves of the split tile 47
                    sync.wait_ge(dve, NT - 1)
                    mid = lo + NFREE
                    sync.dma_start(out_d[:, lo:mid], st_t[:, lo:mid]).then_inc(dout, 16)
                    mid2 = mid + NFREE // 2
                    sync.wait_ge(dve, NT)
                    sync.dma_start(out_d[:, mid:mid2], st_t[:, mid:mid2]).then_inc(dout, 16)
                    sync.wait_ge(dve, NT + 1)
                    split2(out_d, st_t, mid2, hi, dout)
                    ninc += 4
                else:
                    sync.wait_ge(dve, OCH * (c + 1))
                    sync.dma_start(out_d[:, lo:hi], st_t[:, lo:hi]).then_inc(dout, 16)
                    ninc += 1
            sync.wait_ge(dout, 16 * ninc)

        @block.tensor
        def _(tensor):
            # pre-warm the PE HAM clock gate during the initial DMA wait:
            # ~5us of dummy matmuls (zeros into bank 7, which tile 7
            # later clears with start=True) flips the PE to full clock
            # before the real stream begins. st_t is idle SBUF.
            # 12 dummies x ~426ns cold = ~5us: ends about when wave-1 DMA
            # lands, and >3.4us of PE activity flips the clock to 2.4GHz
            tensor.wait_ge(warm, 1)
            for _ in range(11):
                nc.tensor.matmul(
                    ps_t[:, 7 * NFREE:8 * NFREE],
                    st_t[0:1, 0:128],
                    st_t[0:1, 0:NFREE],
                    start=True,
                    stop=True,
                )
            # phase A: xa-only accumulation for tiles 0..PHA-1, gated on the
            # just-in-time xa row chunks
            tensor.wait_ge(dwt1, 32)
            tensor.wait_ge(da0, 32)
            for k in range(3):
                emit_mms(tensor, k, slots_a, start=True, stop=False)
            tensor.wait_ge(da1, 32)
            for k in range(3, PHA):
                emit_mms(tensor, k, slots_a, start=True, stop=False)
            # phase B: finish tiles 0..PHA-1 with the xb/xc pair slots
            tensor.wait_ge(dwt2, 16)
            tensor.wait_ge(db1, 32)
            tensor.wait_ge(dc1, 32)
            for k in range(PHA):
                emit_mms(tensor, k, slots_bc, start=False, stop=True)
            # steady state
            tensor.wait_ge(da2, 32)
            tensor.wait_ge(db2, 32)
            tensor.wait_ge(dc2, 32)
            # one bank-reuse wait covers 4 tiles: tiles k..k+3 need at most
            # dve >= k+3-(NPS-1) = k-4, and DVE lags PE by well under the
            # 3-tile slack this leaves. Fewer waits = fewer PE queue stalls.
            for k in range(PHA, NT - 1):
                if (k - PHA) % 4 == 0:
                    tensor.wait_ge(dve, min(k + 3, NT - 1) - NPS + 1)
                emit_mms(tensor, k, slots_a, start=True, stop=False)
                emit_mms(tensor, k, slots_bc, start=False, stop=True)
            # final tile split into two 2-row groups (N=256 in half banks):
            # the first half's epilogue+DMA overlaps the second half's
            # matmuls, shortening the kernel tail
            k = NT - 1
            m, blk = divmod(k, NBLK)
            j0 = blk * RB
            for h in range(2):
                # halves in DIFFERENT banks (7, then 6): DVE reads half 1
                # while PE accumulates half 2, and same-bank PE-write +
                # DVE-read is a fatal PSUM collision. Bank 6 (tile 46) is
                # free once dve >= NT-1.
                if h == 1:
                    tensor.wait_ge(dve, NT - 1)
                ps_h = ps_t[:, (7 - h) * NFREE:(7 - h) * NFREE + NFREE // 2]
                for sl, is_last in ((slots_a, False), (slots_bc, True)):
                    n = len(sl)
                    for i, (s, view, kh, kw) in enumerate(sl):
                        mm = nc.tensor.matmul(
                            ps_h,
                            wt_t[:, s * COUT + m * 128: s * COUT + (m + 1) * 128],
                            view[:, j0 + 2 * h + kh: j0 + 2 * h + kh + RB // 2,
                                 kw: kw + W],
                            start=(sl is slots_a and i == 0),
                            stop=(is_last and i == n - 1),
                        )
                        if is_last and i == n - 1:
                            mm.then_inc(pes, 1)

        @block.vector
        def _(vector):
            nc.vector.memset(st_t[0:1, 0:NFREE], 0.0).then_inc(warm, 1)
            vector.wait_ge(drest, 48)  # bias + valid resident (3 chunks)
            for k in range(NT - 1):
                m, blk = divmod(k, NBLK)
                ps = ps_t[:, (k % NPS) * NFREE:(k % NPS + 1) * NFREE]
                vector.wait_ge(pes, k + 1)
                nc.vector.scalar_tensor_tensor(
                    st_t[:, k * NFREE:(k + 1) * NFREE],
                    ps,
                    bt_t[:, m:m + 1],
                    vt_t[:, blk * NFREE:(blk + 1) * NFREE],
                    mybir.AluOpType.add,
                    mybir.AluOpType.mult,
                ).then_inc(dve, 1)
            # final tile: two half-width epilogues matching the split groups
            k = NT - 1
            m, blk = divmod(k, NBLK)
            HF = NFREE // 2
            for h in range(2):
                ps_h = ps_t[:, (7 - h) * NFREE:(7 - h) * NFREE + HF]
                vector.wait_ge(pes, k + 1 + h)
                nc.vector.scalar_tensor_tensor(
                    st_t[:, k * NFREE + h * HF:k * NFREE + (h + 1) * HF],
                    ps_h,
                    bt_t[:, m:m + 1],
                    vt_t[:, blk * NFREE + h * HF:blk * NFREE + (h + 1) * HF],
                    mybir.AluOpType.add,
                    mybir.AluOpType.mult,
                ).then_inc(dve, 1)
    return nc


def _causal_mask():
    m = np.ones((KH, KW), dtype=np.float32)
    m[KH // 2, KW // 2:] = 0.0
    m[KH // 2 + 1:, :] = 0.0
    return m


def _prepare_in_maps(x, weight, bias, mask):
    # window-any of mask -> valid [B, H, W] float32
    ind = (np.asarray(mask)[:, 0] != 0)
    indp = np.zeros((B, H + 2 * PAD, W + 2 * PAD), dtype=bool)
    indp[:, PAD:PAD + H, PAD:PAD + W] = ind
    valid = np.zeros((B, H, W), dtype=bool)
    for dh in range(KH):
        for dw in range(KW):
            valid |= indp[:, dh:dh + H, dw:dw + W]
    valid_f = valid.astype(np.float32)

    w_bf = (np.asarray(weight, dtype=np.float32) * _causal_mask()[None, None]).astype(BF16)

    # 18 weight tiles [K=128, M=384] -> one SBUF image [128, 18, 384]
    wt = np.zeros((18, 128, COUT), dtype=BF16)
    for s, (kh, kw) in enumerate(TAPS):
        wt[s] = w_bf[:, 0:128, kh, kw].T
    for i, (ta, tb) in enumerate(PAIRS_XB):
        wt[12 + i, 0:64] = w_bf[:, 128:192, ta[0], ta[1]].T
        wt[12 + i, 64:128] = w_bf[:, 128:192, tb[0], tb[1]].T
    ta, tb = PAIR_XC
    wt[17, 0:64] = w_bf[:, 128:192, ta[0], ta[1]].T
    wt[17, 64:128] = w_bf[:, 128:192, tb[0], tb[1]].T
    wt_sb = np.ascontiguousarray(wt.transpose(1, 0, 2))

    bias_t = np.ascontiguousarray(
        np.asarray(bias, dtype=np.float32).reshape(3, 128).T)

    x_bf = np.asarray(x, dtype=np.float32).astype(BF16)

    in_maps = []
    for c in range(NCORES):
        b, half = c // 2, c % 2
        r0 = half * HHALF
        xp = np.zeros((CIN, NROWS, WP), dtype=BF16)
        lo = r0 - PAD
        src_lo = max(lo, 0)
        xp[:, src_lo - lo:, PAD:PAD + W] = x_bf[b, :, src_lo:r0 + HHALF, :]
        xf = xp.reshape(CIN, FLAT)
        x2 = xf[128:192]
        sh1 = np.zeros_like(x2)
        sh1[:, :-1] = x2[:, 1:]
        shr = np.zeros_like(x2)
        shr[:, :-WP] = x2[:, WP:]
        vrow = valid_f[b, r0:r0 + HHALF].reshape(1, HHALF * W).astype(BF16)
        vt = np.ascontiguousarray(np.broadcast_to(vrow, (128, HHALF * W)))
        in_maps.append({
            "xa": np.ascontiguousarray(xf[0:128]),
            "xb": np.ascontiguousarray(np.concatenate([x2, sh1], axis=0)),
            "xc": np.ascontiguousarray(np.concatenate([x2, shr], axis=0)),
            "wt": wt_sb.reshape(128, 18 * COUT),
            "bt": bias_t,
            "vt": vt,
        })
    return in_maps


def _assemble(results):
    out_full = np.zeros((B, COUT, H, W), dtype=np.float32)
    for c in range(NCORES):
        b, half = c // 2, c % 2
        o = np.asarray(results[c]["out"]).astype(np.float32)
        o4 = o.reshape(128, 3, HHALF, W).transpose(1, 0, 2, 3).reshape(COUT, HHALF, W)
        out_full[b, :, half * HHALF:(half + 1) * HHALF, :] = o4
    return out_full


def kernel(x, weight, bias, mask, _trace=False):
    in_maps = _prepare_in_maps(x, weight, bias, mask)
    nc = _build_program()
    res = run_bass_kernel_spmd(nc, in_maps, core_ids=list(range(NCORES)),
                               trace=_trace)
    out = _assemble(res.results)
    if _trace:
        return out, res
    return out
